# revision 13
# baseline (speedup 1.0000x reference)
"""DSNT + JSD + distance double loss on 8 TRN2 NeuronCores.

Data-parallel: batch 64 -> 8 shards of 8 batches; each core computes its
partial sum s_i over its 16 (b,c) heatmap slices; host sums and /B.

Per (b,c) slice (512x512 -> SBUF [128, 2048], partition p holds rows
h in {4p..4p+3}):
  e    = exp(x) (bf16)     ACT, accum -> rowe [P,1]
  S    = allreduce(rowe)   GpSimd partition_all_reduce -> Sb [P,1]
  invS = recip(Sb)         DVE [P,1]
  cols = [ones;ys]^T @ e   PE [2,512] PSUM; DVE stt xs-dot -> px,py
  p    = e * invS          DVE tensor_scalar bf16 (4x mode)
  m2   = p + t             GpSimd TT (only TT/tcopy/allreduce exist
                           on Pool; TensorScalarPtr is rejected)
  l    = ln(0.5*m2)        ACT scale=0.5 -> = ln(m), folds the ln2
                           term: sum m2*l = sum m2 ln m2 - ln2 sum m2
  w    = sum(m2*l)         PE gram: 16 chunk matmuls m2^T l into a
                           whole-run [128,128] PSUM group; diagonal
                           extracted once in the tail
  sq   : ACT Square[0:1536] accum; GpSimd TT mult [1536:2048] + PE
         colsums (onesb lhsT) -> whole-run [1,512] PSUM
  argmax(t): DVE max8 + max_index into strided accum tiles; exact
  first-occurrence combine in the tail (masked-min on flat index).
jsd_total = [0.5*sum(m2*l) - 0.25*sum(m2^2)] / (H*W)
"""

import math
import os

import numpy as np

import concourse.bacc as bacc
import concourse.bass as bass
import concourse.bass_isa as bass_isa
import concourse.mybir as mybir
import concourse.tile as tile
from concourse.bass_utils import run_bass_kernel_spmd

F32 = mybir.dt.float32
BF16 = mybir.dt.bfloat16
U32 = mybir.dt.uint32
I32 = mybir.dt.int32
ALU = mybir.AluOpType
ACTF = mybir.ActivationFunctionType
AX = mybir.AxisListType

B, C, H, W = 64, 2, 512, 512
N_CORES = 8
B_SH = B // N_CORES          # 8 batches per core
NSL = B_SH * C               # 16 slices per core
P = 128                      # SBUF partitions
FD = (H * W) // P            # 2048 free elements per partition
SUB = W                      # 512-wide sub-columns (4 per row)
NSUB = FD // SUB             # 4

# engine splits (columns, multiples of 512 keep PE chunks clean)
W_DVE = int(os.environ.get("K_W_DVE", "1024"))   # w-mult cols on DVE
SQ_ACT = int(os.environ.get("K_SQ_ACT", "1536"))  # square cols on ACT
# S broadcast: gpsimd partition_all_reduce ("ar") or PE+ACT copies ("pe")
S_BCAST = os.environ.get("K_S_BCAST", "ar")
USE_GRAM = os.environ.get("K_GRAM", "1") == "1"

_CACHE = {}
LAST_RESULTS = None


def _constants():
    # big [128, 138]: [0:8]=oy (col-sum lhsT: col 2j = ones, 2j+1 = ys_j),
    #                 [8]=pbase, [9]=ones, [10:138]=identity
    hidx = (np.arange(P, dtype=np.float32)[:, None] * NSUB
            + np.arange(NSUB, dtype=np.float32)[None, :])
    ys = (hidx + 1.0) / H                                   # [128, 4]
    big = np.zeros((P, 138), dtype=np.float32)
    for j in range(NSUB):
        big[:, 2 * j] = 1.0
        big[:, 2 * j + 1] = ys[:, j]
    big[:, 8] = np.arange(P, dtype=np.float32) * FD
    big[:, 9] = 1.0
    big[:, 10:138] = np.eye(P, dtype=np.float32)
    # small [2, 640]: [:, 0:512] = {xs row, ones row}; [0, 512:640] = onesr
    xs = (np.arange(W, dtype=np.float32) + 1.0) / W
    small = np.zeros((2, 640), dtype=np.float32)
    small[0, 0:W] = xs
    small[1, 0:W] = 1.0
    small[0, W:W + P] = 1.0
    import ml_dtypes
    onesb = np.ones((P, 9), dtype=ml_dtypes.bfloat16)
    for j in range(NSUB):
        onesb[:, 1 + 2 * j] = 1.0
        onesb[:, 2 + 2 * j] = ys[:, j].astype(ml_dtypes.bfloat16)
    return {"big_c": big, "small_c": small, "onesb_c": onesb}


def _patch_act_tables():
    """Steer the act-table chooser so Exp/Ln/Square/Copy all live in the
    single `natural_log_exp_and_others` set — otherwise per-slice
    Exp->Ln alternation reloads tables (~1.3us each)."""
    if _CACHE.get("act_patched"):
        return
    import concourse.hw_specs as hw_specs

    orig = hw_specs.get_activation_tables
    hot = {ACTF.Exp, ACTF.Ln, ACTF.Square, ACTF.Copy, ACTF.Identity}

    def patched(module_arch):
        tabs = orig(module_arch)
        out = {}
        for name, funcs in tabs.items():
            if name == "natural_log_exp_and_others":
                out[name] = set(funcs)
            else:
                out[name] = set(funcs) - hot
        return out

    hw_specs.get_activation_tables = patched
    bacc.get_activation_tables = patched
    _CACHE["act_patched"] = True


def build_program():
    if "nc" in _CACHE:
        return _CACHE["nc"]

    _patch_act_tables()
    nc = bacc.Bacc("TRN2", target_bir_lowering=False, debug=False,
                   num_devices=N_CORES)

    x_d = nc.dram_tensor("x", [NSL, P, FD], F32, kind="ExternalInput").ap()
    t_d = nc.dram_tensor("t", [NSL, P, FD], F32, kind="ExternalInput").ap()
    big_d = nc.dram_tensor("big_c", [P, 138], F32, kind="ExternalInput").ap()
    sml_d = nc.dram_tensor("small_c", [2, 640], F32,
                           kind="ExternalInput").ap()
    onb_d = nc.dram_tensor("onesb_c", [P, 9], BF16,
                       kind="ExternalInput").ap()
    out_d = nc.dram_tensor("out", [1, 1], F32, kind="ExternalOutput").ap()

    with tile.TileContext(nc) as tc:
        _emit(nc, tc, x_d, t_d, big_d, sml_d, onb_d, out_d)

    nc.compile()
    _CACHE["nc"] = nc
    return nc


def _emit(nc, tc, x_d, t_d, big_d, sml_d, onb_d, out_d):
    from contextlib import ExitStack
    ctx = ExitStack()
    with ctx:
        singles = ctx.enter_context(tc.tile_pool(name="singles", bufs=1))
        xp = ctx.enter_context(tc.tile_pool(name="xp", bufs=3))
        tp = ctx.enter_context(tc.tile_pool(name="tp", bufs=3))
        ep = ctx.enter_context(tc.tile_pool(name="ep", bufs=3))
        pp = ctx.enter_context(tc.tile_pool(name="pp", bufs=3))
        m2p = ctx.enter_context(tc.tile_pool(name="m2p", bufs=3))
        lp = ctx.enter_context(tc.tile_pool(name="lp", bufs=3))
        sqap = ctx.enter_context(tc.tile_pool(name="sqap", bufs=2))
        sqgp = ctx.enter_context(tc.tile_pool(name="sqgp", bufs=3))
        sm = ctx.enter_context(tc.tile_pool(name="sm", bufs=4))
        pcols = ctx.enter_context(
            tc.tile_pool(name="pcols", bufs=2, space="PSUM"))
        pgram = ctx.enter_context(
            tc.tile_pool(name="pgram", bufs=1, space="PSUM"))
        psqg = ctx.enter_context(
            tc.tile_pool(name="psqg", bufs=1, space="PSUM"))
        ptail = ctx.enter_context(
            tc.tile_pool(name="ptail", bufs=1, space="PSUM"))

        # ---- constants: 3 packed DMAs on the ACT queue ----
        big_sb = singles.tile([P, 138], F32)
        nc.scalar.dma_start(out=big_sb, in_=big_d)
        sml_sb = singles.tile([2, 640], F32)
        nc.scalar.dma_start(out=sml_sb, in_=sml_d)
        onb_sb = singles.tile([P, 9], BF16)
        nc.scalar.dma_start(out=onb_sb, in_=onb_d)
        onesb_sb = onb_sb[:, 0:1]
        oyb_sb = onb_sb[:, 1:9]
        pb_sb = big_sb[:, 8:9]
        ones_sb = big_sb[:, 9:10]
        eye_sb = big_sb[:, 10:10 + P]
        xo_sb = sml_sb[:, 0:W]
        onesr_sb = sml_sb[0:1, W:W + P]

        # ---- whole-run accumulators ----
        Sb_all = singles.tile([P, NSL], F32)      # S bcast to all partitions
        invSf_all = singles.tile([P, NSL], F32)   # 1/S f32
        sqa_all = singles.tile([P, NSL], F32)     # ACT-part sum(m2^2)
        pxpy_all = singles.tile([2, NSL], F32)
        pmax8_all = singles.tile([P, 8 * NSL], F32)
        ix8_all = singles.tile([P, 8 * NSL], U32)
        rowe_all = singles.tile([P, NSL], F32)    # exp row sums
        if USE_GRAM:
            gram_ps = pgram.tile([P, P], F32, name="gram_ps")
            wsum_ps = None
        else:
            gram_ps = None
            wsum_ps = pgram.tile([1, W], F32, name="wsum_ps")
        sqg_ps = psqg.tile([1, W], F32)           # whole-run gpsimd-sq sums

        x_sb = [None] * NSL
        t_sb = [None] * NSL
        e_sb = [None] * NSL
        m2_sb = [None] * NSL
        l_sb = [None] * NSL

        def stage_a(s):
            x_sb[s] = xp.tile([P, FD], F32, tag="x", name="x_sb")
            nc.sync.dma_start(out=x_sb[s], in_=x_d[s])
            t_sb[s] = tp.tile([P, FD], F32, tag="t", name="t_sb")
            nc.sync.dma_start(out=t_sb[s], in_=t_d[s])
            e_sb[s] = ep.tile([P, FD], BF16, tag="e", name="e_sb")
            nc.scalar.activation(out=e_sb[s], in_=x_sb[s], func=ACTF.Exp,
                                 accum_out=rowe_all[:, s:s + 1])

        def stage_b(s):
            # argmax passes depend only on t(s)
            nc.vector.max(out=pmax8_all[:, 8 * s:8 * s + 8], in_=t_sb[s])
            nc.vector.max_index(out=ix8_all[:, 8 * s:8 * s + 8],
                                in_max=pmax8_all[:, 8 * s:8 * s + 8],
                                in_values=t_sb[s])
            # S (bcast to all partitions) and 1/S
            if S_BCAST == "ar":
                nc.gpsimd.partition_all_reduce(
                    Sb_all[:, s:s + 1], rowe_all[:, s:s + 1],
                    channels=P, reduce_op=bass_isa.ReduceOp.add)
            else:
                s_ps = ptail.tile([1, 1], F32, tag="bc")
                nc.tensor.matmul(s_ps[0:1, 0:1], lhsT=ones_sb[:, 0:1],
                                 rhs=rowe_all[:, s:s + 1],
                                 start=True, stop=True)
                s_row = sm.tile([1, 1], F32, tag="s_row")
                nc.scalar.copy(out=s_row, in_=s_ps[0:1, 0:1])
                sb_ps = ptail.tile([P, 1], F32, tag="tp")
                nc.tensor.matmul(sb_ps[:, 0:1], lhsT=onesr_sb[0:1, :],
                                 rhs=s_row, start=True, stop=True)
                nc.scalar.copy(out=Sb_all[:, s:s + 1], in_=sb_ps[:, 0:1])
            nc.vector.reciprocal(out=invSf_all[:, s:s + 1],
                                 in_=Sb_all[:, s:s + 1])
            # e col sums -> [2,512] PSUM; xs-dot -> px_u,py_u (accum)
            cols2 = pcols.tile([2, W], F32, tag="cols", name="cols2")
            for j in range(NSUB):
                nc.tensor.matmul(
                    cols2[0:2, :], lhsT=oyb_sb[:, 2 * j:2 * j + 2],
                    rhs=e_sb[s][:, j * SUB:(j + 1) * SUB],
                    start=(j == 0), stop=(j == NSUB - 1))
            pxscr = sm.tile([2, W], F32, tag="pxscr")
            nc.vector.scalar_tensor_tensor(
                out=pxscr, in0=cols2[0:2, :], scalar=1.0, in1=xo_sb,
                op0=ALU.mult, op1=ALU.mult,
                accum_out=pxpy_all[0:2, s:s + 1])
            # p = e * invS (bf16 4x), m2 = p + t (gpsimd TT)
            p_sb = pp.tile([P, FD], BF16, tag="p", name="p_sb")
            nc.vector.tensor_scalar_mul(out=p_sb, in0=e_sb[s],
                                        scalar1=invSf_all[:, s:s + 1])
            m2_sb[s] = m2p.tile([P, FD], BF16, tag="m2", name="m2_sb")
            nc.gpsimd.tensor_tensor(out=m2_sb[s], in0=p_sb, in1=t_sb[s],
                                    op=ALU.add)

        def stage_c(s):
            # l = ln(0.5*m2) = ln(m); folds the -ln2*sum(m2) jsd term
            l_sb[s] = lp.tile([P, FD], BF16, tag="l", name="l_sb")
            nc.scalar.activation(out=l_sb[s], in_=m2_sb[s], func=ACTF.Ln,
                                 scale=0.5)
            # squares: ACT part (accum) + gpsimd part (PE colsums)
            sqa_scr = sqap.tile([P, SQ_ACT], BF16, tag="sqa")
            nc.scalar.activation(
                out=sqa_scr, in_=m2_sb[s][:, 0:SQ_ACT], func=ACTF.Square,
                accum_out=sqa_all[:, s:s + 1])
            if SQ_ACT < FD:
                sqg_scr = sqgp.tile([P, FD - SQ_ACT], BF16, tag="sqg")
                nc.gpsimd.tensor_tensor(
                    out=sqg_scr, in0=m2_sb[s][:, SQ_ACT:FD],
                    in1=m2_sb[s][:, SQ_ACT:FD], op=ALU.mult)
                for j, c0 in enumerate(range(SQ_ACT, FD, SUB)):
                    c1 = min(c0 + SUB, FD)
                    nc.tensor.matmul(
                        sqg_ps[0:1, 0:c1 - c0], lhsT=onesb_sb,
                        rhs=sqg_scr[:, c0 - SQ_ACT:c1 - SQ_ACT],
                        start=(s == 0 and j == 0),
                        stop=(s == NSL - 1 and c1 == FD),
                        skip_group_check=True)
            # whole-run gram accumulation: sum(m2 * l) on the PE
            if USE_GRAM:
                for c in range(FD // P):
                    nc.tensor.matmul(
                        gram_ps[:, :], lhsT=m2_sb[s][:, c * P:(c + 1) * P],
                        rhs=l_sb[s][:, c * P:(c + 1) * P],
                        start=(s == 0 and c == 0),
                        stop=(s == NSL - 1 and c == FD // P - 1),
                        skip_group_check=True)
            else:
                w_sb = sqgp.tile([P, FD], BF16, tag="w", name="w_sb")
                nc.vector.tensor_tensor(out=w_sb, in0=m2_sb[s],
                                        in1=l_sb[s], op=ALU.mult)
                for j in range(NSUB):
                    nc.tensor.matmul(
                        wsum_ps[0:1, :], lhsT=onesb_sb,
                        rhs=w_sb[:, j * SUB:(j + 1) * SUB],
                        start=(s == 0 and j == 0),
                        stop=(s == NSL - 1 and j == NSUB - 1),
                        skip_group_check=True)
            x_sb[s] = t_sb[s] = e_sb[s] = None

        def stage_d(s):
            m2_sb[s] = l_sb[s] = None

        stage_a(0)
        stage_a(1)
        for s in range(NSL):
            if s + 2 < NSL:
                stage_a(s + 2)
            stage_b(s)
            if s >= 1:
                stage_c(s - 1)
            if s >= 2:
                stage_d(s - 2)
        stage_c(NSL - 1)

        _emit_tail(nc, tc, singles, sm, ptail, out_d,
                   invSf_all, sqa_all, pxpy_all, pmax8_all, ix8_all,
                   gram_ps, wsum_ps, sqg_ps, pb_sb, ones_sb, eye_sb,
                   onesr_sb)


def _emit_tail(nc, tc, fin, sm, ptail, out_d,
               invSf_all, sqa_all, pxpy_all, pmax8_all, ix8_all,
               gram_ps, wsum_ps, sqg_ps, pb_sb, ones_sb, eye_sb,
               onesr_sb):
    invS = invSf_all[0:1, :]                     # [1,16]

    # ---- grand totals ----
    # Ag = sum(m2*l) = trace of the gram: mask with eye, row-sum, col-sum
    if USE_GRAM:
        gdiag_scr = fin.tile([P, P], F32)
        gdiag = fin.tile([P, 1], F32)
        nc.vector.tensor_tensor_reduce(
            out=gdiag_scr, in0=gram_ps[:, :], in1=eye_sb, scale=1.0,
            scalar=0.0, op0=ALU.mult, op1=ALU.add, accum_out=gdiag)
    else:
        gdiag = fin.tile([P, 1], F32)
        nc.vector.memset(gdiag, 0.0)
        nc.vector.reduce_sum(out=gdiag[0:1, 0:1], in_=wsum_ps[0:1, :],
                             axis=AX.X)
    Cg = fin.tile([1, 1], F32)                   # gpsimd-part sum(m2^2)
    nc.vector.reduce_sum(out=Cg, in_=sqg_ps[0:1, :], axis=AX.X)
    AC_ps = ptail.tile([1, NSL + 1], F32, tag="bc")
    nc.tensor.matmul(AC_ps[0:1, 0:1], lhsT=ones_sb[:, 0:1],
                     rhs=gdiag, start=True, stop=True)
    nc.tensor.matmul(AC_ps[0:1, 1:NSL + 1], lhsT=ones_sb[:, 0:1],
                     rhs=sqa_all, start=True, stop=True)
    ACs = fin.tile([1, NSL + 1], F32)
    nc.vector.tensor_copy(out=ACs, in_=AC_ps[0:1, :])
    Ag = ACs[0:1, 0:1]
    Ca = fin.tile([1, 1], F32)
    nc.vector.reduce_sum(out=Ca, in_=ACs[0:1, 1:NSL + 1], axis=AX.X)

    # ---- px, py ----
    pyu_row = fin.tile([1, NSL], F32)
    nc.sync.dma_start(out=pyu_row, in_=pxpy_all[1:2, :])
    px = fin.tile([1, NSL], F32)
    nc.vector.tensor_tensor(out=px, in0=pxpy_all[0:1, :], in1=invS,
                            op=ALU.mult)
    py = fin.tile([1, NSL], F32)
    nc.vector.tensor_tensor(out=py, in0=pyu_row, in1=invS, op=ALU.mult)

    # ---- exact argmax combine (first occurrence) ----
    pmax_v = pmax8_all[:, :].rearrange("p (s e) -> p s e", e=8)[:, :, 0:1]
    pmax = fin.tile([P, NSL], F32)
    nc.vector.tensor_copy(out=pmax, in_=pmax_v)
    ix_v = ix8_all[:, :].rearrange("p (s e) -> p s e", e=8)[:, :, 0:1]
    ixf = fin.tile([P, NSL], F32)
    nc.vector.tensor_copy(out=ixf, in_=ix_v)    # u32 -> f32
    flat = fin.tile([P, NSL], F32)
    nc.vector.tensor_scalar(out=flat, in0=ixf, scalar1=pb_sb,
                            scalar2=None, op0=ALU.add)

    pmaxT = ptail.tile([NSL, P], F32, tag="tp")
    nc.tensor.transpose(pmaxT[0:NSL, :], pmax, eye_sb)
    m_col = fin.tile([NSL, 1], F32)
    nc.vector.reduce_max(out=m_col, in_=pmaxT[0:NSL, :], axis=AX.X)
    m_row = ptail.tile([1, NSL], F32, tag="bc")
    nc.tensor.transpose(m_row[0:1, :], m_col, eye_sb[0:NSL, 0:NSL])
    m_row_sb = fin.tile([1, NSL], F32)
    nc.vector.tensor_copy(out=m_row_sb, in_=m_row[0:1, :])
    m_rep = ptail.tile([P, NSL], F32, tag="tp")
    nc.tensor.matmul(m_rep[:, :], lhsT=onesr_sb[0:1, :],
                     rhs=m_row_sb, start=True, stop=True)
    mk = fin.tile([P, NSL], F32)
    nc.vector.tensor_tensor(out=mk, in0=pmax, in1=m_rep[:, :],
                            op=ALU.is_lt)
    # first occurrence = min(flat + 1e9*mk) = -max((mk * -1e9) - flat)
    fneg = fin.tile([P, NSL], F32)
    nc.vector.scalar_tensor_tensor(
        out=fneg, in0=mk, scalar=-1.0e9, in1=flat,
        op0=ALU.mult, op1=ALU.subtract)
    fnegT = ptail.tile([NSL, P], F32, tag="tp")
    nc.tensor.transpose(fnegT[0:NSL, :], fneg, eye_sb)
    fmax_col = fin.tile([NSL, 1], F32)
    nc.vector.reduce_max(out=fmax_col, in_=fnegT[0:NSL, :], axis=AX.X)
    fmin_col = fin.tile([NSL, 1], F32)
    nc.vector.tensor_scalar(out=fmin_col, in0=fmax_col, scalar1=-1.0,
                            scalar2=None, op0=ALU.mult)
    f_row = ptail.tile([1, NSL], F32, tag="bc")
    nc.tensor.transpose(f_row[0:1, :], fmin_col, eye_sb[0:NSL, 0:NSL])
    F_sb = fin.tile([1, NSL], F32)
    nc.vector.tensor_copy(out=F_sb, in_=f_row[0:1, :])

    # decompose flat -> (h, w); tx = (w+1)/W, ty = (h+1)/H
    Fi = fin.tile([1, NSL], I32)
    nc.vector.tensor_copy(out=Fi, in_=F_sb)
    wi = fin.tile([1, NSL], I32)
    nc.vector.tensor_scalar(out=wi, in0=Fi, scalar1=W - 1,
                            scalar2=None, op0=ALU.bitwise_and)
    hi = fin.tile([1, NSL], I32)
    nc.vector.tensor_scalar(out=hi, in0=Fi, scalar1=9,
                            scalar2=None, op0=ALU.arith_shift_right)
    wf = fin.tile([1, NSL], F32)
    nc.vector.tensor_copy(out=wf, in_=wi)
    hf = fin.tile([1, NSL], F32)
    nc.vector.tensor_copy(out=hf, in_=hi)
    tx = fin.tile([1, NSL], F32)
    nc.vector.tensor_scalar(out=tx, in0=wf, scalar1=1.0,
                            scalar2=1.0 / W, op0=ALU.add, op1=ALU.mult)
    ty = fin.tile([1, NSL], F32)
    nc.vector.tensor_scalar(out=ty, in0=hf, scalar1=1.0,
                            scalar2=1.0 / H, op0=ALU.add, op1=ALU.mult)

    # ---- ed = sqrt((tx-px)^2 + (ty-py)^2) ----
    dx = fin.tile([1, NSL], F32)
    nc.vector.tensor_tensor(out=dx, in0=tx, in1=px, op=ALU.subtract)
    dy = fin.tile([1, NSL], F32)
    nc.vector.tensor_tensor(out=dy, in0=ty, in1=py, op=ALU.subtract)
    d2 = fin.tile([1, NSL], F32)
    nc.vector.tensor_tensor(out=d2, in0=dx, in1=dx, op=ALU.mult)
    d2b = fin.tile([1, NSL], F32)
    nc.vector.tensor_tensor(out=d2b, in0=dy, in1=dy, op=ALU.mult)
    ed2 = fin.tile([1, NSL], F32)
    nc.vector.tensor_tensor(out=ed2, in0=d2, in1=d2b, op=ALU.add)

    # ---- pair (c=0 vs c=1) distances ----
    NP2 = NSL // 2

    def pairs(v):
        r = v.rearrange("p (b c) -> p b c", c=2)
        return r[:, :, 0:1], r[:, :, 1:2]

    px0, px1 = pairs(px[0:1, :])
    py0, py1 = pairs(py[0:1, :])
    tx0, tx1 = pairs(tx[0:1, :])
    ty0, ty1 = pairs(ty[0:1, :])
    dpx = fin.tile([1, NP2, 1], F32)
    nc.vector.tensor_tensor(out=dpx, in0=px0, in1=px1, op=ALU.subtract)
    dpy = fin.tile([1, NP2, 1], F32)
    nc.vector.tensor_tensor(out=dpy, in0=py0, in1=py1, op=ALU.subtract)
    dtx = fin.tile([1, NP2, 1], F32)
    nc.vector.tensor_tensor(out=dtx, in0=tx0, in1=tx1, op=ALU.subtract)
    dty = fin.tile([1, NP2, 1], F32)
    nc.vector.tensor_tensor(out=dty, in0=ty0, in1=ty1, op=ALU.subtract)
    pd2 = fin.tile([1, NP2, 1], F32)
    nc.vector.tensor_tensor(out=pd2, in0=dpx, in1=dpx, op=ALU.mult)
    pd2b = fin.tile([1, NP2, 1], F32)
    nc.vector.tensor_tensor(out=pd2b, in0=dpy, in1=dpy, op=ALU.mult)
    nc.vector.tensor_tensor(out=pd2, in0=pd2, in1=pd2b, op=ALU.add)
    td2 = fin.tile([1, NP2, 1], F32)
    nc.vector.tensor_tensor(out=td2, in0=dtx, in1=dtx, op=ALU.mult)
    td2b = fin.tile([1, NP2, 1], F32)
    nc.vector.tensor_tensor(out=td2b, in0=dty, in1=dty, op=ALU.mult)
    nc.vector.tensor_tensor(out=td2, in0=td2, in1=td2b, op=ALU.add)

    # sqrts grouped (single act-table switch)
    ed = fin.tile([1, NSL], F32)
    nc.scalar.activation(out=ed, in_=ed2, func=ACTF.Sqrt)
    pd = fin.tile([1, NP2, 1], F32)
    nc.scalar.activation(out=pd, in_=pd2, func=ACTF.Sqrt)
    td = fin.tile([1, NP2, 1], F32)
    nc.scalar.activation(out=td, in_=td2, func=ACTF.Sqrt)

    eds = fin.tile([1, 1], F32)
    nc.vector.reduce_sum(out=eds, in_=ed, axis=AX.X)
    dd = fin.tile([1, NP2, 1], F32)
    nc.vector.tensor_tensor(out=dd, in0=pd, in1=td, op=ALU.subtract)
    dsum = fin.tile([1, 1], F32)
    nc.vector.tensor_reduce(out=dsum, in_=dd, axis=AX.XY, op=ALU.add,
                            apply_absolute_value=True)

    # ---- total = sum(ed) + sum|pd-td| + [0.5*Ag - 0.25*(Ca+Cg)]/N ----
    Cs = fin.tile([1, 1], F32)
    nc.vector.tensor_tensor(out=Cs, in0=Ca, in1=Cg, op=ALU.add)
    j2 = fin.tile([1, 1], F32)
    nc.vector.scalar_tensor_tensor(
        out=j2, in0=Cs, scalar=-0.5, in1=Ag, op0=ALU.mult, op1=ALU.add)
    stot = fin.tile([1, 1], F32)
    nc.vector.scalar_tensor_tensor(
        out=stot, in0=j2, scalar=0.5 / float(H * W), in1=eds,
        op0=ALU.mult, op1=ALU.add)
    nc.vector.tensor_tensor(out=stot, in0=stot, in1=dsum, op=ALU.add)
    nc.sync.dma_start(out=out_d[0:1, 0:1], in_=stot)


def make_in_maps(input, target):
    consts = _constants()
    in_maps = []
    for i in range(N_CORES):
        xs = np.ascontiguousarray(
            input[i * B_SH:(i + 1) * B_SH].reshape(NSL, P, FD))
        ts = np.ascontiguousarray(
            target[i * B_SH:(i + 1) * B_SH].reshape(NSL, P, FD))
        m = {"x": xs, "t": ts}
        m.update(consts)
        in_maps.append(m)
    return in_maps


def kernel(input, target):
    global LAST_RESULTS
    input = np.asarray(input, dtype=np.float32)
    target = np.asarray(target, dtype=np.float32)
    nc = build_program()
    in_maps = make_in_maps(input, target)
    res = run_bass_kernel_spmd(nc, in_maps, list(range(N_CORES)))
    LAST_RESULTS = res
    s = 0.0
    for i in range(N_CORES):
        s += float(res.results[i]["out"][0, 0])
    return np.array([s / B], dtype=np.float32)
